# revision 4
# baseline (speedup 1.0000x reference)
"""Trainium2 Bass kernel for nn_CalibrationLayer (empirical-CDF calibration).

y[i] = piecewise-linear interp of x[i] into (reference_inputs, reference_outputs),
clamped at the table ends — i.e. jnp.searchsorted(ri, x, 'right') + lerp.

Device algorithm (exact, gather-light):
  The calibration function g is piecewise-linear with 4095 interior knots.
  A uniform grid with G=8192 cells over [ri[0], ri[-1]] has <=3 knots per
  cell (verified at build time against the actual table).  Per element:
     xc = clamp(x, lo, hi)
     k  = floor((xc-lo)/w)            (float-bit trick, exact)
     g(xc) = C_k + S_k*xc + sum_i a_i(k)*relu(xc - ts_i(k))   (i = 1..3)
  (C,S,ts1,a1) and (ts2,a2,ts3,a3) are gathered from per-partition SBUF
  tables with the GPSIMD ap_gather instruction (two phases, since each
  8192x4 f32 table is 128KB/partition).

Sharding: data-parallel over 8 NeuronCores; tables replicated.
"""

import numpy as np

import concourse.bacc as bacc
import concourse.mybir as mybir
from concourse.tile import TileContext
from concourse.bass_utils import run_bass_kernel_spmd
from concourse.alu_op_type import AluOpType

f32 = mybir.dt.float32
i32 = mybir.dt.int32
i16 = mybir.dt.int16

BATCH = 8388608
R = 4096
N_CORES = 8
N_PER_CORE = BATCH // N_CORES          # 1048576
COLS = N_PER_CORE // 128               # 8192 columns per partition
C_SUB = 128                            # columns per chunk
N_CHUNKS = COLS // C_SUB               # 64
G = 8192                               # uniform grid cells
BIG = np.float32(3.0e38)               # pad knot: relu(x - BIG) == 0

_cache = {}
_last_exec_ns = [None]


def last_exec_time_ns():
    return _last_exec_ns[0]


def _build_tables(ri, ro):
    """Host-side: grid tables from the runtime calibration table (f64 math)."""
    ri64 = ri.astype(np.float64)
    ro64 = ro.astype(np.float64)
    lo64, hi64 = ri64[0], ri64[-1]
    w64 = (hi64 - lo64) / G

    # segment j (1..R-1) covers [ri[j-1], ri[j]]:  y = C64[j] + S64[j]*x
    S64 = np.zeros(R, np.float64)
    C64 = np.zeros(R, np.float64)
    S64[1:] = (ro64[1:] - ro64[:-1]) / (ri64[1:] - ri64[:-1])
    C64[1:] = ro64[:-1] - S64[1:] * ri64[:-1]

    # device cell map fuzz: u = fl(fl(xc*inv32)+B32) vs exact; widen cells
    fz = 0.02 * w64

    edges = lo64 + w64 * np.arange(G + 1)
    lo_e = edges[:-1] - fz
    hi_e = edges[1:] + fz

    # j_left[k]: segment valid just above cell-left (widened)
    jl = np.clip(np.searchsorted(ri64, lo_e, side="right"), 1, R - 1)

    # interior knots m=1..R-2 (slope change a_m = S[m+1]-S[m] at ri[m])
    km = np.arange(1, R - 1)
    a64 = S64[km + 1] - S64[km]
    # first knot index strictly above lo_e for each cell
    m0 = np.searchsorted(ri64[1:R - 1], lo_e, side="right") + 1  # in [1, R-1]

    TA = np.zeros((G, 4), np.float32)
    TB = np.zeros((G, 4), np.float32)
    TA[:, 0] = C64[jl]
    TA[:, 1] = S64[jl]
    TA[:, 2] = BIG
    TB[:, 0] = BIG
    TB[:, 2] = BIG

    max_knots = 0
    for k in range(G):
        m = m0[k]
        cnt = 0
        vals = []
        while m <= R - 2 and ri64[m] < hi_e[k]:
            vals.append((np.float32(ri64[m]), np.float32(a64[m - 1])))
            m += 1
            cnt += 1
        max_knots = max(max_knots, cnt)
        if cnt > 3:
            raise AssertionError(f"cell {k} has {cnt} knots; grid too coarse")
        if cnt >= 1:
            TA[k, 2], TA[k, 3] = vals[0]
        if cnt >= 2:
            TB[k, 0], TB[k, 1] = vals[1]
        if cnt >= 3:
            TB[k, 2], TB[k, 3] = vals[2]

    inv32 = np.float32(G / (hi64 - lo64))
    B32 = np.float32(8192.0 - lo64 * (G / (hi64 - lo64)))
    return TA, TB, np.float32(lo64), np.float32(hi64), inv32, B32


def _relu_terms(nc, pool, xc, ex3, col0, col1, y_acc):
    """y_acc += ex3[:,:,col1] * relu(xc - ex3[:,:,col0])  (in place)."""
    r = pool.tile([128, C_SUB], f32, tag="rt")
    nc.vector.tensor_tensor(r[:], xc[:], ex3[:, :, col0], AluOpType.subtract)
    nc.vector.tensor_scalar(r[:], r[:], 0.0, None, AluOpType.max)
    nc.vector.tensor_tensor(r[:], r[:], ex3[:, :, col1], AluOpType.mult)
    nc.vector.tensor_tensor(y_acc[:], y_acc[:], r[:], AluOpType.add)


def _phase(nc, tc, x_d, tab_tile, in_y_d, out_y_d, lo, hi, inv, B, phase_a):
    with tc.tile_pool(name=f"ph{int(phase_a)}", bufs=2) as pool, \
         tc.tile_pool(name=f"go{int(phase_a)}", bufs=1) as gpool:
        for ch in range(N_CHUNKS):
            sl = slice(ch * C_SUB, (ch + 1) * C_SUB)
            x = pool.tile([128, C_SUB], f32, tag="x")
            nc.sync.dma_start(x[:], x_d[:, sl])

            xc = pool.tile([128, C_SUB], f32, tag="xc")
            nc.vector.tensor_scalar(xc[:], x[:], float(lo), float(hi),
                                    AluOpType.max, AluOpType.min)
            u = pool.tile([128, C_SUB], f32, tag="u")
            nc.vector.tensor_scalar(u[:], xc[:], float(inv), float(B),
                                    AluOpType.mult, AluOpType.add)
            k32 = pool.tile([128, C_SUB], i32, tag="k32")
            nc.vector.tensor_scalar(k32[:], u[:].bitcast(i32), 10, None,
                                    AluOpType.logical_shift_right)
            nc.vector.tensor_scalar(k32[:], k32[:], 0x118000, 0,
                                    AluOpType.subtract, AluOpType.max)
            nc.vector.tensor_scalar(k32[:], k32[:], G - 1, None, AluOpType.min)
            k16 = pool.tile([128, C_SUB], i16, tag="k16")
            nc.vector.tensor_copy(k16[:], k32[:])

            gout = gpool.tile([128, 16 * C_SUB * 4], f32, tag="gout")
            nc.gpsimd.ap_gather(
                gout[:].rearrange("p (s v) -> p s v", v=4),
                tab_tile[:].rearrange("p (g v) -> p g v", v=4),
                k16[:],
                channels=128, num_elems=G, d=4, num_idxs=16 * C_SUB,
            )
            ex = pool.tile([128, C_SUB * 4], f32, tag="ex")
            g3 = gout[:].rearrange("p (s v) -> p s v", v=4)
            ex3 = ex[:].rearrange("p (c v) -> p c v", v=4)
            for r in range(16):
                nc.sync.dma_start(ex3[r:128:16, :, :], g3[r:128:16, r::16, :])

            y = pool.tile([128, C_SUB], f32, tag="y")
            if phase_a:
                # y = C + S*xc + a1*relu(xc-ts1)
                nc.vector.tensor_tensor(y[:], xc[:], ex3[:, :, 1], AluOpType.mult)
                nc.vector.tensor_tensor(y[:], y[:], ex3[:, :, 0], AluOpType.add)
                _relu_terms(nc, pool, xc, ex3, 2, 3, y)
            else:
                # y = y1 + a2*relu(xc-ts2) + a3*relu(xc-ts3)
                nc.sync.dma_start(y[:], in_y_d[:, sl])
                _relu_terms(nc, pool, xc, ex3, 0, 1, y)
                _relu_terms(nc, pool, xc, ex3, 2, 3, y)
            nc.sync.dma_start(out_y_d[:, sl], y[:])


def _build_kernel(lo, hi, inv, B):
    nc = bacc.Bacc(target_bir_lowering=False)
    with TileContext(nc) as tc:
        x_d = nc.dram_tensor("x", [128, COLS], f32, kind="ExternalInput")
        ta_d = nc.dram_tensor("ta", [G * 4], f32, kind="ExternalInput")
        tb_d = nc.dram_tensor("tb", [G * 4], f32, kind="ExternalInput")
        y1_d = nc.dram_tensor("y1", [128, COLS], f32, kind="Internal")
        y_d = nc.dram_tensor("y", [128, COLS], f32, kind="ExternalOutput")

        with tc.tile_pool(name="tab", bufs=1) as tpool:
            tab = tpool.tile([128, G * 4], f32, tag="tab")
            nc.sync.dma_start(tab[:], ta_d[:].partition_broadcast(128))
            _phase(nc, tc, x_d, tab, None, y1_d, lo, hi, inv, B, True)
            nc.sync.dma_start(tab[:], tb_d[:].partition_broadcast(128))
            _phase(nc, tc, x_d, tab, y1_d, y_d, lo, hi, inv, B, False)
    nc.finalize()
    return nc


def kernel(x, reference_inputs, reference_outputs):
    x = np.asarray(x, dtype=np.float32)
    ri = np.asarray(reference_inputs, dtype=np.float32)
    ro = np.asarray(reference_outputs, dtype=np.float32)
    assert x.shape == (BATCH, 1) and ri.shape == (R,) and ro.shape == (R,)

    TA, TB, lo, hi, inv, B = _build_tables(ri, ro)

    key = (float(lo), float(hi), float(inv), float(B))
    if key not in _cache:
        _cache[key] = _build_kernel(lo, hi, inv, B)
    nc = _cache[key]

    shards = x[:, 0].reshape(N_CORES, 128, COLS)
    in_maps = [
        {"x": shards[i], "ta": TA.reshape(-1), "tb": TB.reshape(-1)}
        for i in range(N_CORES)
    ]
    import os
    trace = bool(os.environ.get("KERNEL_TRACE"))
    res = run_bass_kernel_spmd(nc, in_maps, core_ids=list(range(N_CORES)),
                               trace=trace)
    if res.exec_time_ns is not None:
        _last_exec_ns[0] = res.exec_time_ns
    out = np.stack([r["y"] for r in res.results])  # [8, 128, COLS]
    return out.reshape(BATCH, 1)


# revision 8
# speedup vs baseline: 6.9136x; 6.9136x over previous
"""Trainium2 Bass kernel for nn_CalibrationLayer (empirical-CDF calibration).

y[i] = piecewise-linear interp of x[i] into (reference_inputs, reference_outputs),
clamped at the table ends — i.e. jnp.searchsorted(ri, x, 'right') + lerp.

Device algorithm (exact, gather-light):
  The calibration function g is piecewise-linear with 4095 interior knots.
  A uniform grid with G=8192 cells over [ri[0], ri[-1]] has <=3 knots per
  cell (verified at build time against the actual table).  Per element:
     xc = clamp(x, lo, hi)
     k  = floor((xc-lo)/w)            (float-bit trick, exact)
     g(xc) = C_k + S_k*xc + sum_i a_i(k)*relu(xc - ts_i(k))   (i = 1..3)
  (C,S,ts1,a1) and (ts2,a2,ts3,a3) are gathered from per-partition SBUF
  tables with the GPSIMD ap_gather instruction (two phases, since each
  8192x4 f32 table is 128KB/partition).

Sharding: data-parallel over 8 NeuronCores; tables replicated.
"""

import os

import numpy as np

import concourse.bacc as bacc
import concourse.mybir as mybir
from concourse.tile import TileContext
from concourse.bass_utils import run_bass_kernel_spmd
from concourse.alu_op_type import AluOpType

f32 = mybir.dt.float32
i32 = mybir.dt.int32
i16 = mybir.dt.int16

BATCH = 8388608
R = 4096
N_CORES = 8
N_PER_CORE = BATCH // N_CORES          # 1048576
COLS = N_PER_CORE // 128               # 8192 columns per partition
C_SUB = 128                            # columns per chunk
N_CHUNKS = COLS // C_SUB               # 64
G = 8192                               # uniform grid cells
BIG = np.float32(3.0e38)               # pad knot: relu(x - BIG) == 0

_cache = {}
_last_exec_ns = [None]


def last_exec_time_ns():
    return _last_exec_ns[0]


def _build_tables(ri, ro):
    """Host-side: grid tables from the runtime calibration table (f64 math)."""
    ri64 = ri.astype(np.float64)
    ro64 = ro.astype(np.float64)
    lo64, hi64 = ri64[0], ri64[-1]
    w64 = (hi64 - lo64) / G

    # segment j (1..R-1) covers [ri[j-1], ri[j]]:  y = C64[j] + S64[j]*x
    S64 = np.zeros(R, np.float64)
    C64 = np.zeros(R, np.float64)
    S64[1:] = (ro64[1:] - ro64[:-1]) / (ri64[1:] - ri64[:-1])
    C64[1:] = ro64[:-1] - S64[1:] * ri64[:-1]

    # device cell map fuzz: u = fl(fl(xc*inv32)+B32) vs exact; widen cells
    fz = 0.02 * w64

    edges = lo64 + w64 * np.arange(G + 1)
    lo_e = edges[:-1] - fz
    hi_e = edges[1:] + fz

    # j_left[k]: segment valid just above cell-left (widened)
    jl = np.clip(np.searchsorted(ri64, lo_e, side="right"), 1, R - 1)

    # interior knots m=1..R-2 (slope change a_m = S[m+1]-S[m] at ri[m])
    km = np.arange(1, R - 1)
    a64 = S64[km + 1] - S64[km]
    # first knot index strictly above lo_e for each cell
    m0 = np.searchsorted(ri64[1:R - 1], lo_e, side="right") + 1  # in [1, R-1]

    TA = np.zeros((G, 4), np.float32)
    TB = np.zeros((G, 4), np.float32)
    TA[:, 0] = C64[jl]
    TA[:, 1] = S64[jl]
    TA[:, 2] = BIG
    TB[:, 0] = BIG
    TB[:, 2] = BIG

    max_knots = 0
    for k in range(G):
        m = m0[k]
        cnt = 0
        vals = []
        while m <= R - 2 and ri64[m] < hi_e[k]:
            vals.append((np.float32(ri64[m]), np.float32(a64[m - 1])))
            m += 1
            cnt += 1
        max_knots = max(max_knots, cnt)
        if cnt > 3:
            raise AssertionError(f"cell {k} has {cnt} knots; grid too coarse")
        if cnt >= 1:
            TA[k, 2], TA[k, 3] = vals[0]
        if cnt >= 2:
            TB[k, 0], TB[k, 1] = vals[1]
        if cnt >= 3:
            TB[k, 2], TB[k, 3] = vals[2]

    inv32 = np.float32(G / (hi64 - lo64))
    B32 = np.float32(8192.0 - lo64 * (G / (hi64 - lo64)))
    return TA, TB, np.float32(lo64), np.float32(hi64), inv32, B32


def _relu_terms(nc, pool, xc, ex3, col0, col1, y_acc):
    """y_acc += ex3[:,:,col1] * relu(xc - ex3[:,:,col0])  (in place)."""
    r = pool.tile([128, C_SUB], f32, tag="rt")
    nc.vector.tensor_tensor(r[:], xc[:], ex3[:, :, col0], AluOpType.subtract)
    nc.vector.tensor_scalar(r[:], r[:], 0.0, None, AluOpType.max)
    nc.vector.tensor_tensor(r[:], r[:], ex3[:, :, col1], AluOpType.mult)
    nc.vector.tensor_tensor(y_acc[:], y_acc[:], r[:], AluOpType.add)


def _phase(nc, tc, x_d, tab_tile, in_y_d, out_y_d, lo, hi, inv, B, phase_a):
    with tc.tile_pool(name=f"ph{int(phase_a)}", bufs=2) as pool, \
         tc.tile_pool(name=f"go{int(phase_a)}", bufs=1) as gpool:
        for ch in range(N_CHUNKS):
            sl = slice(ch * C_SUB, (ch + 1) * C_SUB)
            x = pool.tile([128, C_SUB], f32, tag="x")
            nc.sync.dma_start(x[:], x_d[:, sl])

            xc = pool.tile([128, C_SUB], f32, tag="xc")
            nc.vector.tensor_scalar(xc[:], x[:], float(lo), float(hi),
                                    AluOpType.max, AluOpType.min)
            u = pool.tile([128, C_SUB], f32, tag="u")
            nc.vector.tensor_scalar(u[:], xc[:], float(inv), float(B),
                                    AluOpType.mult, AluOpType.add)
            k32 = pool.tile([128, C_SUB], i32, tag="k32")
            nc.vector.tensor_scalar(k32[:], u[:].bitcast(i32), 10, None,
                                    AluOpType.logical_shift_right)
            nc.vector.tensor_scalar(k32[:], k32[:], 0x118000, 0,
                                    AluOpType.subtract, AluOpType.max)
            nc.vector.tensor_scalar(k32[:], k32[:], G - 1, None, AluOpType.min)
            k16 = pool.tile([128, C_SUB], i16, tag="k16")
            nc.vector.tensor_copy(k16[:], k32[:])

            gout = gpool.tile([128, 16 * C_SUB * 4], f32, tag="gout")
            nc.gpsimd.ap_gather(
                gout[:].rearrange("p (s v) -> p s v", v=4),
                tab_tile[:].rearrange("p (g v) -> p g v", v=4),
                k16[:],
                channels=128, num_elems=G, d=4, num_idxs=16 * C_SUB,
            )
            ex = pool.tile([128, C_SUB * 4], f32, tag="ex")
            g3 = gout[:].rearrange("p (s v) -> p s v", v=4)
            ex3 = ex[:].rearrange("p (c v) -> p c v", v=4)
            for r in range(16):
                nc.sync.dma_start(ex3[r:128:16, :, :], g3[r:128:16, r::16, :])

            y = pool.tile([128, C_SUB], f32, tag="y")
            if phase_a:
                # y = C + S*xc + a1*relu(xc-ts1)
                nc.vector.tensor_tensor(y[:], xc[:], ex3[:, :, 1], AluOpType.mult)
                nc.vector.tensor_tensor(y[:], y[:], ex3[:, :, 0], AluOpType.add)
                _relu_terms(nc, pool, xc, ex3, 2, 3, y)
            else:
                # y = y1 + a2*relu(xc-ts2) + a3*relu(xc-ts3)
                nc.sync.dma_start(y[:], in_y_d[:, sl])
                _relu_terms(nc, pool, xc, ex3, 0, 1, y)
                _relu_terms(nc, pool, xc, ex3, 2, 3, y)
            nc.sync.dma_start(out_y_d[:, sl], y[:])


def _build_kernel(lo, hi, inv, B):
    nc = bacc.Bacc(target_bir_lowering=False)
    with TileContext(nc) as tc:
        x_d = nc.dram_tensor("x", [128, COLS], f32, kind="ExternalInput")
        ta_d = nc.dram_tensor("ta", [G * 4], f32, kind="ExternalInput")
        tb_d = nc.dram_tensor("tb", [G * 4], f32, kind="ExternalInput")
        y1_d = nc.dram_tensor("y1", [128, COLS], f32, kind="Internal")
        y_d = nc.dram_tensor("y", [128, COLS], f32, kind="ExternalOutput")

        with tc.tile_pool(name="tab", bufs=1) as tpool:
            tab = tpool.tile([128, G * 4], f32, tag="tab")
            nc.sync.dma_start(tab[:], ta_d[:].partition_broadcast(128))
            _phase(nc, tc, x_d, tab, None, y1_d, lo, hi, inv, B, True)
            nc.sync.dma_start(tab[:], tb_d[:].partition_broadcast(128))
            _phase(nc, tc, x_d, tab, y1_d, y_d, lo, hi, inv, B, False)
    nc.finalize()
    return nc


def _build_memcpy_kernel():
    """x -> y via SBUF, for timing baseline (framework + transfer overhead)."""
    nc = bacc.Bacc(target_bir_lowering=False)
    with TileContext(nc) as tc:
        x_d = nc.dram_tensor("x", [128, COLS], f32, kind="ExternalInput")
        ta_d = nc.dram_tensor("ta", [G * 4], f32, kind="ExternalInput")
        tb_d = nc.dram_tensor("tb", [G * 4], f32, kind="ExternalInput")
        y_d = nc.dram_tensor("y", [128, COLS], f32, kind="ExternalOutput")
        with tc.tile_pool(name="mtab", bufs=1) as tpool, \
             tc.tile_pool(name="m", bufs=4) as pool:
            t0 = tpool.tile([128, G * 4], f32, tag="tabs")
            nc.sync.dma_start(t0[:], ta_d[:].partition_broadcast(128))
            nc.sync.dma_start(t0[:], tb_d[:].partition_broadcast(128))
            for ch in range(0, COLS, 2048):
                t = pool.tile([128, 2048], f32, tag="t")
                nc.sync.dma_start(t[:], x_d[:, ch:ch + 2048])
                nc.sync.dma_start(y_d[:, ch:ch + 2048], t[:])
    nc.finalize()
    return nc


def memcpy_kernel(x, reference_inputs, reference_outputs):
    """Timing baseline: same I/O contract, device does only DMA."""
    x = np.asarray(x, dtype=np.float32)
    if "memcpy" not in _cache:
        _cache["memcpy"] = _build_memcpy_kernel()
    nc = _cache["memcpy"]
    shards = x[:, 0].reshape(N_CORES, 128, COLS)
    z = np.zeros(G * 4, np.float32)
    in_maps = [{"x": shards[i], "ta": z, "tb": z} for i in range(N_CORES)]
    res = run_bass_kernel_spmd(nc, in_maps, core_ids=list(range(N_CORES)))
    return np.stack([r["y"] for r in res.results]).reshape(BATCH, 1)


def kernel(x, reference_inputs, reference_outputs):
    x = np.asarray(x, dtype=np.float32)
    ri = np.asarray(reference_inputs, dtype=np.float32)
    ro = np.asarray(reference_outputs, dtype=np.float32)
    assert x.shape == (BATCH, 1) and ri.shape == (R,) and ro.shape == (R,)

    TA, TB, lo, hi, inv, B = _build_tables(ri, ro)

    key = (float(lo), float(hi), float(inv), float(B))
    if key not in _cache:
        _cache[key] = _build_kernel(lo, hi, inv, B)
    nc = _cache[key]

    shards = x[:, 0].reshape(N_CORES, 128, COLS)
    in_maps = [
        {"x": shards[i], "ta": TA.reshape(-1), "tb": TB.reshape(-1)}
        for i in range(N_CORES)
    ]
    trace = bool(os.environ.get("KERNEL_TRACE"))
    res = run_bass_kernel_spmd(nc, in_maps, core_ids=list(range(N_CORES)),
                               trace=trace)
    if res.exec_time_ns is not None:
        _last_exec_ns[0] = res.exec_time_ns
    out = np.stack([r["y"] for r in res.results])  # [8, 128, COLS]
    return out.reshape(BATCH, 1)
